# revision 11
# baseline (speedup 1.0000x reference)
"""Trainium2 Bass kernel for nn_BaseAttention (causal MHA, b=2, n=2048, d=1024, 16 heads).

Sharding (8 cores): core c handles batch c//4 and heads 4*(c%4)..4*(c%4)+3.
- W_q/W_k/W_v column-sharded (256 cols/core), W_o row-sharded (256 rows/core).
- Each core computes a partial output [2048, 1024] in fp32; host sums the 4
  partials per batch (row-parallel out-projection) and stacks the 2 batches.

V2 schedule (ACT-paced): the exp() activations (80 x [128,1024], ~1.1us each on
the scalar engine) are the long pole of the attention phase, so the kernel is
organized to start them as early as possible and keep the exp pipeline fed:
  - only the head-pair-0 Q^T/K^T projection chains for q-tile 0 run up front;
    everything else (V, head-pair-1 Q/K, next round's projections, early
    out-projection chunks) is woven as PE filler inside the attention rounds.
  - per k-chunk the S^T scores for the even/odd head go into ONE shared psum
    tile (even head cols 0:512, odd head cols 512:1024, different banks), so
    both 64-partition row-tiled S matmuls become ready at the same time and
    execute concurrently in the PE array (2x S throughput), and one exp
    covers both heads.
  - out-projection chunks for q-tile j run as filler in round j+1 (they only
    need round j's normalized ctx), spreading the output DMA across the run.
  - weights are staged t-major so the first Q chain needs only the first
    256KB weight DMA; x arrives in (c-half, row-group) pieces so the first
    chain starts as soon as ~0.5MB has landed.
"""
import sys, types

sys.path.insert(0, "/opt/trn_rl_repo")


def _install_ntff_shim():
    # antenv.axon_hooks is absent in this image; register the NTFF profile
    # hook via ctypes so run_bass_kernel_spmd(trace=True) works under axon.
    if "antenv.axon_hooks" in sys.modules:
        return
    try:
        sys.path.insert(0, "/root/.axon_site")
        from trn_agent_boot.trn_boot import _ntff_profile_via_ctypes

        hook = _ntff_profile_via_ctypes("/opt/axon/libaxon_pjrt.so")
        mod = types.ModuleType("antenv.axon_hooks")
        mod.get_axon_ntff_profile_hook = lambda: hook
        mod.set_axon_ntff_profile_hook = lambda h: None
        sys.modules["antenv.axon_hooks"] = mod
    except Exception:
        pass


_install_ntff_shim()

import numpy as np
import ml_dtypes
import concourse.bass as bass
import concourse.mybir as mybir
import concourse.tile as tile
from concourse import bacc
from concourse.bass_utils import run_bass_kernel_spmd
from contextlib import ExitStack

f32 = mybir.dt.float32
bf16 = mybir.dt.bfloat16
EXP = mybir.ActivationFunctionType.Exp

SEQ = 2048          # sequence length
DIN = 1024          # model dim (8 chunks of 128)
QC = 256            # q/k/v cols per core (4 heads x 64)
HD = 64             # head dim
NH = 4              # heads per core
NG = 4              # row groups of 512
VST = NH * 65       # Vones stride per row chunk (4 heads x (64 V + 1 ones))

TRACE = False
LAST_RESULTS = None


def build_nc():
    nc = bacc.Bacc()
    x_d = nc.dram_tensor("x", [DIN, SEQ], bf16, kind="ExternalInput")  # pre-transposed on host
    wq_d = nc.dram_tensor("wq", [128, 8 * QC], bf16, kind="ExternalInput")  # t-major
    wk_d = nc.dram_tensor("wk", [128, 8 * QC], bf16, kind="ExternalInput")  # t-major
    wv_d = nc.dram_tensor("wv", [128, 8 * QC], bf16, kind="ExternalInput")  # c-major
    wo_d = nc.dram_tensor("wo", [128, 2 * DIN], bf16, kind="ExternalInput")
    bo_d = nc.dram_tensor("bo", [1, DIN], bf16, kind="ExternalInput")
    out_d = nc.dram_tensor("out", [SEQ, DIN], f32, kind="ExternalOutput")

    with tile.TileContext(nc, pool_alloc_mode="queue") as tc, ExitStack() as ctx:
        cst = ctx.enter_context(tc.tile_pool(name="cst", bufs=1))
        wr = ctx.enter_context(tc.tile_pool(name="wr", bufs=1))
        big = ctx.enter_context(tc.tile_pool(name="big", bufs=1))
        ptp = ctx.enter_context(tc.tile_pool(name="ptp", bufs=8))
        nrm = ctx.enter_context(tc.tile_pool(name="nrm", bufs=3))
        ob = ctx.enter_context(tc.tile_pool(name="ob", bufs=6))
        ps = ctx.enter_context(tc.tile_pool(name="ps", bufs=1, space="PSUM"))

        # ---- DMAs ordered by first use. Weights on the scalar HWDGE queue,
        # x + output on sync: the queues issue in parallel.
        # tiny bias DMA first so the gpsimd bias broadcast clears the FIFO
        # before any affine_select masks queue behind it
        bo_sb = cst.tile([1, DIN], bf16)
        nc.scalar.dma_start(bo_sb[:], bo_d[:])
        bo_f = cst.tile([1, DIN], f32)
        nc.vector.tensor_copy(bo_f[:], bo_sb[:])
        bias_bc = cst.tile([128, DIN], f32)
        nc.gpsimd.partition_broadcast(bias_bc[:], bo_f[:])

        # The first exp is gated by wq-t0 + wk-t0 + all of x rows 0:512, so
        # those three lead the sync queue (which starts pumping first); the
        # rest of the weights go on the scalar queue ordered by first use.
        wq_sb = wr.tile([128, 8 * QC], bf16, name="wq_sb")
        wk_sb = wr.tile([128, 8 * QC], bf16, name="wk_sb")
        wv_sb = wr.tile([128, 8 * QC], bf16, name="wv_sb")
        xT = big.tile([128, 8 * SEQ], bf16)
        xview = xT[:].rearrange("p (c r) -> p c r", r=SEQ)
        dview = x_d.rearrange("(c p) r -> p c r", p=128)
        nc.sync.dma_start(wq_sb[:, 0:1024], wq_d[:, 0:1024])
        nc.sync.dma_start(wk_sb[:, 0:1024], wk_d[:, 0:1024])
        nc.sync.dma_start(xview[:, 0:4, 0:512], dview[:, 0:4, 0:512])
        nc.sync.dma_start(xview[:, 4:8, 0:512], dview[:, 4:8, 0:512])
        nc.scalar.dma_start(wv_sb[:], wv_d[:])
        nc.scalar.dma_start(wq_sb[:, 1024:], wq_d[:, 1024:])
        nc.scalar.dma_start(wk_sb[:, 1024:], wk_d[:, 1024:])
        wo_sb = cst.tile([128, 2 * DIN], bf16)
        nc.scalar.dma_start(wo_sb[:], wo_d[:])
        for g in range(1, NG):
            nc.sync.dma_start(
                xview[:, :, g * 512:(g + 1) * 512],
                dview[:, :, g * 512:(g + 1) * 512],
            )

        # ---- persistent activations ----
        qt_sb = [big.tile([128, SEQ], bf16, name=f"qt{t}") for t in range(2)]
        kt_sb = [big.tile([128, SEQ], bf16, name=f"kt{t}") for t in range(2)]
        vones = big.tile([128, 16 * VST], bf16)
        ctxt = [big.tile([128, SEQ], bf16, name=f"ctxt{t}") for t in range(2)]

        vview = vones.rearrange("p (r h e) -> p r h e", h=NH, e=65)
        nc.vector.memset(vview[:, :, :, 64], 1.0)

        # ---- emission helpers ----
        def emit_qk(g, t, wt, dst):
            prj = ps.tile([128, 512], f32, tag="b", bufs=2, name="prj")
            for c in range(8):
                nc.tensor.matmul(
                    prj[:],
                    wt[:, t * 1024 + c * 128: t * 1024 + c * 128 + 128],
                    xT[:, c * SEQ + g * 512: c * SEQ + g * 512 + 512],
                    start=(c == 0),
                    stop=(c == 7),
                )
            nc.vector.tensor_copy(dst[t][:, g * 512:(g + 1) * 512], prj[:])

        def emit_v(g, rc):
            rcg = 4 * g + rc
            vps = ps.tile([128, 256], f32, tag="b", bufs=2, name="vps")
            for c in range(8):
                nc.tensor.matmul(
                    vps[:],
                    xT[:, c * SEQ + rcg * 128: c * SEQ + rcg * 128 + 128],
                    wv_sb[:, c * QC:(c + 1) * QC],
                    start=(c == 0),
                    stop=(c == 7),
                )
            nc.vector.tensor_copy(
                vview[:, rcg, :, 0:64],
                vps[:].rearrange("p (h e) -> p h e", e=HD),
            )

        def emit_outproj(rc, n, tag="b"):
            ops = ps.tile([128, 512], f32, tag=tag, bufs=2, name="ops")
            for u in range(2):
                nc.tensor.matmul(
                    ops[:],
                    ctxt[u][:, rc * 128:(rc + 1) * 128],
                    wo_sb[:, u * DIN + n * 512: u * DIN + n * 512 + 512],
                    start=(u == 0),
                    stop=(u == 1),
                )
            osb = ob.tile([128, 512], f32, tag="o", name="osb")
            nc.vector.tensor_add(osb[:], ops[:], bias_bc[:, n * 512:(n + 1) * 512])
            nc.sync.dma_start(
                out_d[rc * 128:(rc + 1) * 128, n * 512:(n + 1) * 512], osb[:]
            )

        # ---- up-front: only the head-pair-0 Q/K chains for q-tile 0,
        # interleaved c-by-c so both chains pace with the arriving x pieces
        # and finish together.
        prjq = ps.tile([128, 512], f32, tag="b", bufs=2, name="prjq")
        prjk = ps.tile([128, 512], f32, tag="b", bufs=2, name="prjk")
        for c in range(8):
            for wt, prj in ((wq_sb, prjq), (wk_sb, prjk)):
                nc.tensor.matmul(
                    prj[:],
                    wt[:, c * 128: c * 128 + 128],
                    xT[:, c * SEQ: c * SEQ + 512],
                    start=(c == 0),
                    stop=(c == 7),
                )
        nc.vector.tensor_copy(qt_sb[0][:, 0:512], prjq[:])
        nc.vector.tensor_copy(kt_sb[0][:, 0:512], prjk[:])

        # ---- main rounds: attention(j) woven with deferred projections,
        # next round's Q/K, and out-projection of earlier rounds.
        for j in range(NG):
            npair = 2 * j + 2
            # filler list for this round, ordered by needed-by time:
            #  - V(j) (its chunks 4j..4j+3 feed this round's AVs from ip 2j+1
            #    on; for j=0 that is almost immediately, so V goes first then)
            #  - Qt1/Kt1(j) (needed at this round's u=1 phase; emitted early
            #    for j>=1 so the u0->u1 handoff never stalls the exp pipeline)
            #  - Qt0/Kt0(j+1) (needed at next round's start)
            #  - out-projection chunks of round j-1 (ctx normalized last round)
            vj = [lambda rc=rc: emit_v(j, rc) for rc in range(4)]
            qk1 = [
                lambda: emit_qk(j, 1, wq_sb, qt_sb),
                lambda: emit_qk(j, 1, wk_sb, kt_sb),
            ]
            filler = vj + qk1 if j == 0 else qk1 + vj
            n_u1_gate = len(filler)  # fillers that must precede u=1's S
            if j < NG - 1:
                filler += [
                    lambda g=j + 1: emit_qk(g, 0, wq_sb, qt_sb),
                    lambda g=j + 1: emit_qk(g, 0, wk_sb, kt_sb),
                ]
            if j >= 1:
                filler += [
                    (lambda rc=rc, n=n: emit_outproj(rc, n))
                    for rc in range(4 * (j - 1), 4 * j)
                    for n in range(2)
                ]
            steps_total = 4 * npair   # 2 halves per ip, 2 u phases
            fill_i = 0
            step = 0

            for u in range(2):           # head pair u: heads 2u, 2u+1
                if u == 1:
                    # u=1's S needs qt/kt head-pair-1 of this round: make sure
                    # those chains (and this round's V) are already emitted.
                    while fill_i < n_u1_gate:
                        filler[fill_i]()
                        fill_i += 1
                avs = [ps.tile([65, 512], f32, tag="av", bufs=2, name=f"av{p}")
                       for p in range(2)]
                pts = []                 # per ip: (ptX, ptY) bf16 exp tiles

                def emit_s(ip, half, sps):
                    # concurrent row-tiled S pair: even head -> cols 0:512,
                    # odd head -> cols 512:1024 (different psum banks); both
                    # wait on the same exp of the recycled buffer, so they
                    # become ready together and overlap in the PE array.
                    i = 2 * ip + half
                    off = max(0, 128 * i - 512 * j)
                    for p in range(2):
                        o = p * 64
                        nc.tensor.matmul(
                            sps[:, p * 512 + off:(p + 1) * 512],
                            kt_sb[u][o:o + 64, i * 128:(i + 1) * 128],
                            qt_sb[u][o:o + 64, j * 512 + off:(j + 1) * 512],
                            start=True,
                            stop=True,
                        )
                    pt = ptp.tile([128, 1024], bf16, tag="pt", name="pt")
                    nc.scalar.activation(pt[:], sps[:], EXP, scale=0.125)
                    if i >= 4 * j:
                        off = 128 * i - 512 * j
                        for p in range(2):
                            nc.gpsimd.affine_select(
                                out=pt[:, p * 512 + off:(p + 1) * 512],
                                in_=pt[:, p * 512 + off:(p + 1) * 512],
                                compare_op=mybir.AluOpType.is_ge,
                                fill=0.0,
                                base=0,
                                channel_multiplier=-1,
                                pattern=[[1, 512 - off]],
                            )
                    return pt

                def emit_av(kp, half, stop=False):
                    k = 2 * kp + half
                    off = max(0, 128 * k - 512 * j)
                    for p in range(2):
                        h = 2 * u + p
                        nc.tensor.matmul(
                            avs[p][:, off:512],
                            vones[:, k * VST + h * 65: k * VST + h * 65 + 65],
                            pts[kp][half][:, p * 512 + off:(p + 1) * 512],
                            start=(k == 0),
                            stop=stop,
                        )

                for ip in range(npair):
                    # per half: S pair, then the previous chunk's AV pair and
                    # half the filler quota — so the PE never head-of-line
                    # blocks on the second half's exp with ready work behind.
                    cur = [ps.tile([128, 1024], f32, tag="a", bufs=2, name="sps")
                           for _ in range(2)]
                    curpt = []
                    for half in range(2):
                        curpt.append(emit_s(ip, half, cur[half]))
                        if ip >= 1:
                            emit_av(ip - 1, half)
                        step += 1
                        want = (len(filler) * step) // steps_total
                        while fill_i < want:
                            filler[fill_i]()
                            fill_i += 1
                    pts.append(curpt)
                # tail AVs for the last pair + immediate per-parity normalize
                kp = npair - 1
                for half in range(2):
                    emit_av(kp, half, stop=(half == 1))
                for p in range(2):
                    o = p * 64
                    rsrow = nrm.tile([1, 512], f32, tag="rsrow", name="rsrow")
                    nc.vector.tensor_copy(rsrow[:], avs[p][64:65, :])
                    rinv = nrm.tile([1, 512], f32, tag="rinv", name="rinv")
                    nc.vector.reciprocal_approx_fast(rinv[:], rsrow[:])
                    bcast = nrm.tile([64, 512], f32, tag="bcast", name="bcast")
                    nc.gpsimd.partition_broadcast(bcast[:], rinv[:])
                    nc.vector.tensor_mul(
                        ctxt[u][o:o + 64, j * 512:(j + 1) * 512],
                        avs[p][0:64, :],
                        bcast[:],
                    )
            while fill_i < len(filler):
                filler[fill_i]()
                fill_i += 1

        # ---- final out-projection chunks rc12..15, two-pass: all eight
        # u=0 matmuls run during the last head-pair's normalize chain
        # (ctxt[0] is ready before ctxt[1]) — this also keeps the PE warm so
        # the tail matmuls run at full clock — then u=1 + a wide drain.
        # The S-score psum banks (tag "a") are free after the last exp, so
        # rc12/rc13 use wide [128,1024] tiles holding both n-halves each.
        held = []   # (rc, psum-AP covering n0|n1, width)
        for rc in (12, 13):
            ops = ps.tile([128, 1024], f32, tag="a", bufs=2, name="opsw")
            held.append((rc, ops))
        for rc in (14, 15):
            ops0 = ps.tile([128, 512], f32, tag="b", bufs=2, name="ops")
            ops1 = ps.tile([128, 512], f32, tag="av", bufs=2, name="ops")
            held.append((rc, (ops0, ops1)))
        for u in range(2):
            for rc, ops in held:
                for n in range(2):
                    dst = ops[:, n * 512:(n + 1) * 512] if not isinstance(ops, tuple) \
                        else ops[n][:]
                    nc.tensor.matmul(
                        dst,
                        ctxt[u][:, rc * 128:(rc + 1) * 128],
                        wo_sb[:, u * DIN + n * 512: u * DIN + n * 512 + 512],
                        start=(u == 0),
                        stop=(u == 1),
                    )
            if u == 1:
                # drain each chunk as soon as its u=1 accumulation stops
                for rc, ops in held:
                    if isinstance(ops, tuple):
                        for n in range(2):
                            osb = ob.tile([128, 512], f32, tag="o", name="osb")
                            nc.vector.tensor_add(
                                osb[:], ops[n][:],
                                bias_bc[:, n * 512:(n + 1) * 512])
                            nc.sync.dma_start(
                                out_d[rc * 128:(rc + 1) * 128,
                                      n * 512:(n + 1) * 512], osb[:])
                    else:
                        osb = ob.tile([128, 1024], f32, tag="o", name="osbw")
                        nc.vector.tensor_add(osb[:], ops[:], bias_bc[:])
                        nc.sync.dma_start(
                            out_d[rc * 128:(rc + 1) * 128, :], osb[:])

    nc.compile()
    return nc


_NC = None


def _get_nc():
    global _NC
    if _NC is None:
        _NC = build_nc()
    return _NC


def kernel(x, W_q, W_k, W_v, W_o, b_o):
    global LAST_RESULTS
    nc = _get_nc()
    bf = ml_dtypes.bfloat16
    x = np.asarray(x, np.float32).astype(bf)
    # pre-transpose per batch (shared by the 4 cores of each batch)
    xT = [np.ascontiguousarray(x[bi].T) for bi in range(2)]
    W_q = np.asarray(W_q, np.float32).astype(bf)
    W_k = np.asarray(W_k, np.float32).astype(bf)
    W_v = np.asarray(W_v, np.float32).astype(bf)
    W_o = np.asarray(W_o, np.float32).astype(bf)
    b_o = np.asarray(b_o, np.float32).astype(bf).reshape(1, DIN)
    zeros_bo = np.zeros((1, DIN), bf)

    def lay_w_t(w, sl):  # [1024, 256] shard -> t-major [128, 2048]
        # t[p, t*1024 + c*128 + m] = w[c*128+p, sl][t*128+m]
        return np.ascontiguousarray(
            w[:, sl].reshape(8, 128, 2, 128).transpose(1, 2, 0, 3).reshape(128, 8 * QC))

    def lay_w(w, sl):   # [1024, 256] shard -> c-major [128, 8*256]
        return np.ascontiguousarray(
            w[:, sl].reshape(8, 128, QC).transpose(1, 0, 2).reshape(128, 8 * QC))

    def lay_wo(w, sl):  # [256, 1024] shard -> [128, 2*1024]
        return np.ascontiguousarray(
            w[sl, :].reshape(2, 128, DIN).transpose(1, 0, 2).reshape(128, 2 * DIN))

    in_maps = []
    for c in range(8):
        bi, g = c // 4, c % 4
        sl = slice(g * QC, (g + 1) * QC)
        in_maps.append({
            "x": xT[bi],
            "wq": lay_w_t(W_q, sl),
            "wk": lay_w_t(W_k, sl),
            "wv": lay_w(W_v, sl),
            "wo": lay_wo(W_o, sl),
            "bo": b_o if g == 0 else zeros_bo,
        })

    res = run_bass_kernel_spmd(nc, in_maps, list(range(8)), trace=TRACE)
    LAST_RESULTS = res
    outs = [np.asarray(r["out"], dtype=np.float32) for r in res.results]
    return np.stack([
        outs[0] + outs[1] + outs[2] + outs[3],
        outs[4] + outs[5] + outs[6] + outs[7],
    ])


if __name__ == "__main__":
    if "--compile-only" in sys.argv:
        import tempfile
        from concourse.bass_utils import compile_bass_kernel

        nc = build_nc()
        with tempfile.TemporaryDirectory() as td:
            print("walrus compiling...")
            neff = compile_bass_kernel(nc, td)
            print("COMPILE OK", neff)


# revision 17
# speedup vs baseline: 1.0076x; 1.0076x over previous
"""Trainium2 Bass kernel for nn_BaseAttention (causal MHA, b=2, n=2048, d=1024, 16 heads).

Sharding (8 cores): core c handles batch c//4 and heads 4*(c%4)..4*(c%4)+3.
- W_q/W_k/W_v column-sharded (256 cols/core), W_o row-sharded (256 rows/core).
- Each core computes a partial output [2048, 1024] in fp32; host sums the 4
  partials per batch (row-parallel out-projection) and stacks the 2 batches.

V2 schedule (ACT-paced): the exp() activations (80 x [128,1024], ~1.1us each on
the scalar engine) are the long pole of the attention phase, so the kernel is
organized to start them as early as possible and keep the exp pipeline fed:
  - only the head-pair-0 Q^T/K^T projection chains for q-tile 0 run up front;
    everything else (V, head-pair-1 Q/K, next round's projections, early
    out-projection chunks) is woven as PE filler inside the attention rounds.
  - per k-chunk the S^T scores for the even/odd head go into ONE shared psum
    tile (even head cols 0:512, odd head cols 512:1024, different banks), so
    both 64-partition row-tiled S matmuls become ready at the same time and
    execute concurrently in the PE array (2x S throughput), and one exp
    covers both heads.
  - out-projection chunks for q-tile j run as filler in round j+1 (they only
    need round j's normalized ctx), spreading the output DMA across the run.
  - weights are staged t-major so the first Q chain needs only the first
    256KB weight DMA; x arrives in (c-half, row-group) pieces so the first
    chain starts as soon as ~0.5MB has landed.
"""
import sys, types

sys.path.insert(0, "/opt/trn_rl_repo")


def _install_ntff_shim():
    # antenv.axon_hooks is absent in this image; register the NTFF profile
    # hook via ctypes so run_bass_kernel_spmd(trace=True) works under axon.
    if "antenv.axon_hooks" in sys.modules:
        return
    try:
        sys.path.insert(0, "/root/.axon_site")
        from trn_agent_boot.trn_boot import _ntff_profile_via_ctypes

        hook = _ntff_profile_via_ctypes("/opt/axon/libaxon_pjrt.so")
        mod = types.ModuleType("antenv.axon_hooks")
        mod.get_axon_ntff_profile_hook = lambda: hook
        mod.set_axon_ntff_profile_hook = lambda h: None
        sys.modules["antenv.axon_hooks"] = mod
    except Exception:
        pass


_install_ntff_shim()

import numpy as np
import ml_dtypes
import concourse.bass as bass
import concourse.mybir as mybir
import concourse.tile as tile
from concourse import bacc
from concourse.bass_utils import run_bass_kernel_spmd
from contextlib import ExitStack

f32 = mybir.dt.float32
bf16 = mybir.dt.bfloat16
EXP = mybir.ActivationFunctionType.Exp

SEQ = 2048          # sequence length
DIN = 1024          # model dim (8 chunks of 128)
QC = 256            # q/k/v cols per core (4 heads x 64)
HD = 64             # head dim
NH = 4              # heads per core
NG = 4              # row groups of 512
VST = NH * 65       # Vones stride per row chunk (4 heads x (64 V + 1 ones))

TRACE = False
LAST_RESULTS = None


def build_nc():
    nc = bacc.Bacc()
    x_d = nc.dram_tensor("x", [DIN, SEQ], bf16, kind="ExternalInput")  # pre-transposed on host
    wq_d = nc.dram_tensor("wq", [128, 8 * QC], bf16, kind="ExternalInput")  # t-major
    wk_d = nc.dram_tensor("wk", [128, 8 * QC], bf16, kind="ExternalInput")  # t-major
    wv_d = nc.dram_tensor("wv", [128, 8 * QC], bf16, kind="ExternalInput")  # c-major
    wo_d = nc.dram_tensor("wo", [128, 2 * DIN], bf16, kind="ExternalInput")
    bo_d = nc.dram_tensor("bo", [1, DIN], bf16, kind="ExternalInput")
    out_d = nc.dram_tensor("out", [SEQ, DIN], f32, kind="ExternalOutput")

    with tile.TileContext(nc, pool_alloc_mode="queue") as tc, ExitStack() as ctx:
        cst = ctx.enter_context(tc.tile_pool(name="cst", bufs=1))
        wr = ctx.enter_context(tc.tile_pool(name="wr", bufs=1))
        big = ctx.enter_context(tc.tile_pool(name="big", bufs=1))
        ptp = ctx.enter_context(tc.tile_pool(name="ptp", bufs=8))
        nrm = ctx.enter_context(tc.tile_pool(name="nrm", bufs=3))
        ob = ctx.enter_context(tc.tile_pool(name="ob", bufs=6))
        ps = ctx.enter_context(tc.tile_pool(name="ps", bufs=1, space="PSUM"))

        # ---- DMAs ordered by first use. Weights on the scalar HWDGE queue,
        # x + output on sync: the queues issue in parallel.
        # tiny bias DMA first so the gpsimd bias broadcast clears the FIFO
        # before any affine_select masks queue behind it
        bo_sb = cst.tile([1, DIN], bf16)
        nc.scalar.dma_start(bo_sb[:], bo_d[:])
        bo_f = cst.tile([1, DIN], f32)
        nc.vector.tensor_copy(bo_f[:], bo_sb[:])
        bias_bc = cst.tile([128, DIN], f32)
        nc.gpsimd.partition_broadcast(bias_bc[:], bo_f[:])

        # Per-queue DMA bandwidth is only ~170-260 GB/s; the two HWDGE queues
        # together reach ~320. The first exp is gated by wq-t0 + wk-t0 + all
        # of x rows 0:512 (2.5MB), so that critical prefix is split evenly
        # across BOTH queues, as is each later x row-group.
        wq_sb = wr.tile([128, 8 * QC], bf16, name="wq_sb")
        wk_sb = wr.tile([128, 8 * QC], bf16, name="wk_sb")
        wv_sb = wr.tile([128, 8 * QC], bf16, name="wv_sb")
        wo_sb = cst.tile([128, 2 * DIN], bf16)
        xT = big.tile([128, 8 * SEQ], bf16)
        xview = xT[:].rearrange("p (c r) -> p c r", r=SEQ)
        dview = x_d.rearrange("(c p) r -> p c r", p=128)
        nc.sync.dma_start(wq_sb[:, 0:1024], wq_d[:, 0:1024])
        nc.sync.dma_start(xview[:, 0:4, 0:512], dview[:, 0:4, 0:512])
        nc.scalar.dma_start(wk_sb[:, 0:1024], wk_d[:, 0:1024])
        nc.scalar.dma_start(xview[:, 4:8, 0:512], dview[:, 4:8, 0:512])
        nc.scalar.dma_start(wv_sb[:], wv_d[:])
        nc.sync.dma_start(xview[:, 0:4, 512:1024], dview[:, 0:4, 512:1024])
        nc.scalar.dma_start(xview[:, 4:8, 512:1024], dview[:, 4:8, 512:1024])
        nc.sync.dma_start(wq_sb[:, 1024:], wq_d[:, 1024:])
        nc.sync.dma_start(wk_sb[:, 1024:], wk_d[:, 1024:])
        nc.sync.dma_start(wo_sb[:], wo_d[:])
        for g in range(2, NG):
            nc.sync.dma_start(
                xview[:, 0:4, g * 512:(g + 1) * 512],
                dview[:, 0:4, g * 512:(g + 1) * 512],
            )
            nc.scalar.dma_start(
                xview[:, 4:8, g * 512:(g + 1) * 512],
                dview[:, 4:8, g * 512:(g + 1) * 512],
            )

        # ---- persistent activations ----
        qt_sb = [big.tile([128, SEQ], bf16, name=f"qt{t}") for t in range(2)]
        kt_sb = [big.tile([128, SEQ], bf16, name=f"kt{t}") for t in range(2)]
        vones = big.tile([128, 16 * VST], bf16)
        ctxt = [big.tile([128, SEQ], bf16, name=f"ctxt{t}") for t in range(2)]

        vview = vones.rearrange("p (r h e) -> p r h e", h=NH, e=65)
        nc.vector.memset(vview[:, :, :, 64], 1.0)

        # ---- emission helpers ----
        def emit_qk(g, t, wt, dst):
            prj = ps.tile([128, 512], f32, tag="b", bufs=2, name="prj")
            for c in range(8):
                nc.tensor.matmul(
                    prj[:],
                    wt[:, t * 1024 + c * 128: t * 1024 + c * 128 + 128],
                    xT[:, c * SEQ + g * 512: c * SEQ + g * 512 + 512],
                    start=(c == 0),
                    stop=(c == 7),
                )
            nc.vector.tensor_copy(dst[t][:, g * 512:(g + 1) * 512], prj[:])

        def emit_v(g, rc):
            rcg = 4 * g + rc
            vps = ps.tile([128, 256], f32, tag="b", bufs=2, name="vps")
            for c in range(8):
                nc.tensor.matmul(
                    vps[:],
                    xT[:, c * SEQ + rcg * 128: c * SEQ + rcg * 128 + 128],
                    wv_sb[:, c * QC:(c + 1) * QC],
                    start=(c == 0),
                    stop=(c == 7),
                )
            nc.vector.tensor_copy(
                vview[:, rcg, :, 0:64],
                vps[:].rearrange("p (h e) -> p h e", e=HD),
            )

        def emit_outproj(rc, n, tag="b"):
            ops = ps.tile([128, 512], f32, tag=tag, bufs=2, name="ops")
            for u in range(2):
                nc.tensor.matmul(
                    ops[:],
                    ctxt[u][:, rc * 128:(rc + 1) * 128],
                    wo_sb[:, u * DIN + n * 512: u * DIN + n * 512 + 512],
                    start=(u == 0),
                    stop=(u == 1),
                )
            osb = ob.tile([128, 512], f32, tag="o", name="osb")
            nc.vector.tensor_add(osb[:], ops[:], bias_bc[:, n * 512:(n + 1) * 512])
            nc.sync.dma_start(
                out_d[rc * 128:(rc + 1) * 128, n * 512:(n + 1) * 512], osb[:]
            )

        # ---- up-front: only the head-pair-0 Q/K chains for q-tile 0,
        # interleaved c-by-c so both chains pace with the arriving x pieces
        # and finish together.
        prjq = ps.tile([128, 512], f32, tag="b", bufs=2, name="prjq")
        prjk = ps.tile([128, 512], f32, tag="b", bufs=2, name="prjk")
        for c in range(8):
            for wt, prj in ((wq_sb, prjq), (wk_sb, prjk)):
                nc.tensor.matmul(
                    prj[:],
                    wt[:, c * 128: c * 128 + 128],
                    xT[:, c * SEQ: c * SEQ + 512],
                    start=(c == 0),
                    stop=(c == 7),
                )
        nc.vector.tensor_copy(qt_sb[0][:, 0:512], prjq[:])
        nc.vector.tensor_copy(kt_sb[0][:, 0:512], prjk[:])

        # ---- main rounds: attention(j) woven with deferred projections,
        # next round's Q/K, and out-projection of earlier rounds.
        for j in range(NG):
            npair = 2 * j + 2
            # filler list for this round, ordered by needed-by time:
            #  - V(j) (its chunks 4j..4j+3 feed this round's AVs from ip 2j+1
            #    on; for j=0 that is almost immediately, so V goes first then)
            #  - Qt1/Kt1(j) (needed at this round's u=1 phase; emitted early
            #    for j>=1 so the u0->u1 handoff never stalls the exp pipeline)
            #  - Qt0/Kt0(j+1) (needed at next round's start)
            #  - out-projection chunks of round j-1 (ctx normalized last round)
            vj = [lambda rc=rc: emit_v(j, rc) for rc in range(4)]
            qk1 = [
                lambda: emit_qk(j, 1, wq_sb, qt_sb),
                lambda: emit_qk(j, 1, wk_sb, kt_sb),
            ]
            filler = vj + qk1 if j == 0 else qk1 + vj
            n_u1_gate = len(filler)  # fillers that must precede u=1's S
            if j < NG - 1:
                filler += [
                    lambda g=j + 1: emit_qk(g, 0, wq_sb, qt_sb),
                    lambda g=j + 1: emit_qk(g, 0, wk_sb, kt_sb),
                ]
            if j >= 1:
                filler += [
                    (lambda rc=rc, n=n: emit_outproj(rc, n))
                    for rc in range(4 * (j - 1), 4 * j)
                    for n in range(2)
                ]
            steps_total = 4 * npair   # 2 halves per ip, 2 u phases
            if j == 0:
                # hand-paced: V(0) chunks feed this round's AVs almost
                # immediately, and Qt1/Kt1 must finish inside u=0 so the
                # u0->u1 handoff doesn't stall the exp pipeline.
                wants = [1, 2, 4, 6, 7, 8, 8, 8]
            else:
                wants = [(len(filler) * s) // steps_total
                         for s in range(1, steps_total + 1)]
            fill_i = 0
            step = 0

            for u in range(2):           # head pair u: heads 2u, 2u+1
                if u == 1:
                    # u=1's S needs qt/kt head-pair-1 of this round: make sure
                    # those chains (and this round's V) are already emitted.
                    while fill_i < n_u1_gate:
                        filler[fill_i]()
                        fill_i += 1
                avs = [ps.tile([65, 512], f32, tag="av", bufs=2, name=f"av{p}")
                       for p in range(2)]
                pts = []                 # per ip: (ptX, ptY) bf16 exp tiles

                def emit_s(ip, half, sps):
                    # concurrent row-tiled S pair: even head -> cols 0:512,
                    # odd head -> cols 512:1024 (different psum banks); both
                    # wait on the same exp of the recycled buffer, so they
                    # become ready together and overlap in the PE array.
                    i = 2 * ip + half
                    off = max(0, 128 * i - 512 * j)
                    for p in range(2):
                        o = p * 64
                        nc.tensor.matmul(
                            sps[:, p * 512 + off:(p + 1) * 512],
                            kt_sb[u][o:o + 64, i * 128:(i + 1) * 128],
                            qt_sb[u][o:o + 64, j * 512 + off:(j + 1) * 512],
                            start=True,
                            stop=True,
                        )
                    pt = ptp.tile([128, 1024], bf16, tag="pt", name="pt")
                    # exp over the valid causal columns only: a 2-run strided
                    # AP [128, 2 heads, 512-off] skips the fully-masked
                    # [0:off) prefix of each head's half for diagonal chunks.
                    src = sps[:].rearrange("p (v c) -> p v c", v=2)[:, :, off:512]
                    dst = pt[:].rearrange("p (v c) -> p v c", v=2)[:, :, off:512]
                    nc.scalar.activation(dst, src, EXP, scale=0.125)
                    if i >= 4 * j:
                        # one mask for both heads: iota = -ch + 0*v + e >= 0
                        nc.gpsimd.affine_select(
                            out=dst,
                            in_=dst,
                            compare_op=mybir.AluOpType.is_ge,
                            fill=0.0,
                            base=0,
                            channel_multiplier=-1,
                            pattern=[[0, 2], [1, 512 - off]],
                        )
                    return pt

                def emit_av(kp, half, stop=False):
                    k = 2 * kp + half
                    off = max(0, 128 * k - 512 * j)
                    for p in range(2):
                        h = 2 * u + p
                        nc.tensor.matmul(
                            avs[p][:, off:512],
                            vones[:, k * VST + h * 65: k * VST + h * 65 + 65],
                            pts[kp][half][:, p * 512 + off:(p + 1) * 512],
                            start=(k == 0),
                            stop=stop,
                        )

                for ip in range(npair):
                    # per half: S pair, then the previous chunk's AV pair and
                    # half the filler quota — so the PE never head-of-line
                    # blocks on the second half's exp with ready work behind.
                    cur = [ps.tile([128, 1024], f32, tag="a", bufs=2, name="sps")
                           for _ in range(2)]
                    curpt = []
                    for half in range(2):
                        curpt.append(emit_s(ip, half, cur[half]))
                        if ip >= 1:
                            emit_av(ip - 1, half)
                        want = wants[step]
                        step += 1
                        while fill_i < want:
                            filler[fill_i]()
                            fill_i += 1
                    pts.append(curpt)
                # tail AVs for the last pair + immediate per-parity normalize
                kp = npair - 1
                for half in range(2):
                    emit_av(kp, half, stop=(half == 1))
                for p in range(2):
                    o = p * 64
                    rsrow = nrm.tile([1, 512], f32, tag="rsrow", name="rsrow")
                    nc.vector.tensor_copy(rsrow[:], avs[p][64:65, :])
                    rinv = nrm.tile([1, 512], f32, tag="rinv", name="rinv")
                    nc.vector.reciprocal_approx_fast(rinv[:], rsrow[:])
                    bcast = nrm.tile([64, 512], f32, tag="bcast", name="bcast")
                    nc.gpsimd.partition_broadcast(bcast[:], rinv[:])
                    nc.vector.tensor_mul(
                        ctxt[u][o:o + 64, j * 512:(j + 1) * 512],
                        avs[p][0:64, :],
                        bcast[:],
                    )
            while fill_i < len(filler):
                filler[fill_i]()
                fill_i += 1

        # ---- final out-projection chunks rc12..15, two-pass: all eight
        # u=0 matmuls run during the last head-pair's normalize chain
        # (ctxt[0] is ready before ctxt[1]) — this also keeps the PE warm so
        # the tail matmuls run at full clock — then u=1 + a wide drain.
        # The S-score psum banks (tag "a") are free after the last exp, so
        # rc12/rc13 use wide [128,1024] tiles holding both n-halves each.
        held = []   # (rc, psum-AP covering n0|n1, width)
        for rc in (12, 13):
            ops = ps.tile([128, 1024], f32, tag="a", bufs=2, name="opsw")
            held.append((rc, ops))
        for rc in (14, 15):
            ops0 = ps.tile([128, 512], f32, tag="b", bufs=2, name="ops")
            ops1 = ps.tile([128, 512], f32, tag="av", bufs=2, name="ops")
            held.append((rc, (ops0, ops1)))
        for u in range(2):
            for rc, ops in held:
                for n in range(2):
                    dst = ops[:, n * 512:(n + 1) * 512] if not isinstance(ops, tuple) \
                        else ops[n][:]
                    nc.tensor.matmul(
                        dst,
                        ctxt[u][:, rc * 128:(rc + 1) * 128],
                        wo_sb[:, u * DIN + n * 512: u * DIN + n * 512 + 512],
                        start=(u == 0),
                        stop=(u == 1),
                    )
            if u == 1:
                # drain each chunk as soon as its u=1 accumulation stops
                for rc, ops in held:
                    if isinstance(ops, tuple):
                        for n in range(2):
                            osb = ob.tile([128, 512], f32, tag="o", name="osb")
                            nc.vector.tensor_add(
                                osb[:], ops[n][:],
                                bias_bc[:, n * 512:(n + 1) * 512])
                            nc.sync.dma_start(
                                out_d[rc * 128:(rc + 1) * 128,
                                      n * 512:(n + 1) * 512], osb[:])
                    else:
                        osb = ob.tile([128, 1024], f32, tag="o", name="osbw")
                        nc.vector.tensor_add(osb[:], ops[:], bias_bc[:])
                        nc.sync.dma_start(
                            out_d[rc * 128:(rc + 1) * 128, :], osb[:])

    nc.compile()
    return nc


_NC = None


def _get_nc():
    global _NC
    if _NC is None:
        _NC = build_nc()
    return _NC


def kernel(x, W_q, W_k, W_v, W_o, b_o):
    global LAST_RESULTS
    nc = _get_nc()
    bf = ml_dtypes.bfloat16
    x = np.asarray(x, np.float32).astype(bf)
    # pre-transpose per batch (shared by the 4 cores of each batch)
    xT = [np.ascontiguousarray(x[bi].T) for bi in range(2)]
    W_q = np.asarray(W_q, np.float32).astype(bf)
    W_k = np.asarray(W_k, np.float32).astype(bf)
    W_v = np.asarray(W_v, np.float32).astype(bf)
    W_o = np.asarray(W_o, np.float32).astype(bf)
    b_o = np.asarray(b_o, np.float32).astype(bf).reshape(1, DIN)
    zeros_bo = np.zeros((1, DIN), bf)

    def lay_w_t(w, sl):  # [1024, 256] shard -> t-major [128, 2048]
        # t[p, t*1024 + c*128 + m] = w[c*128+p, sl][t*128+m]
        return np.ascontiguousarray(
            w[:, sl].reshape(8, 128, 2, 128).transpose(1, 2, 0, 3).reshape(128, 8 * QC))

    def lay_w(w, sl):   # [1024, 256] shard -> c-major [128, 8*256]
        return np.ascontiguousarray(
            w[:, sl].reshape(8, 128, QC).transpose(1, 0, 2).reshape(128, 8 * QC))

    def lay_wo(w, sl):  # [256, 1024] shard -> [128, 2*1024]
        return np.ascontiguousarray(
            w[sl, :].reshape(2, 128, DIN).transpose(1, 0, 2).reshape(128, 2 * DIN))

    in_maps = []
    for c in range(8):
        bi, g = c // 4, c % 4
        sl = slice(g * QC, (g + 1) * QC)
        in_maps.append({
            "x": xT[bi],
            "wq": lay_w_t(W_q, sl),
            "wk": lay_w_t(W_k, sl),
            "wv": lay_w(W_v, sl),
            "wo": lay_wo(W_o, sl),
            "bo": b_o if g == 0 else zeros_bo,
        })

    res = run_bass_kernel_spmd(nc, in_maps, list(range(8)), trace=TRACE)
    LAST_RESULTS = res
    outs = [np.asarray(r["out"], dtype=np.float32) for r in res.results]
    return np.stack([
        outs[0] + outs[1] + outs[2] + outs[3],
        outs[4] + outs[5] + outs[6] + outs[7],
    ])


if __name__ == "__main__":
    if "--compile-only" in sys.argv:
        import tempfile
        from concourse.bass_utils import compile_bass_kernel

        nc = build_nc()
        with tempfile.TemporaryDirectory() as td:
            print("walrus compiling...")
            neff = compile_bass_kernel(nc, td)
            print("COMPILE OK", neff)
